# revision 59
# baseline (speedup 1.0000x reference)
"""Trainium2 Bass kernel for nn_Attention_9431748182617.

Quirky attention: scores z[b,k,q] = (q_h . k_h) / sqrt(D), softmax over the
QUERY axis (per key row), out[q] = sum_k A[k,q] * v[k], then output projection.

Sharding (8 NeuronCores):
  - tensor-parallel over heads: 16 heads -> 2 heads per core.
    Each core owns rows [128c, 128c+128) of Wq/Wk/Wv (its 2 heads) and
    computes q/k/v + attention for those heads over the full batch.
  - z^T (local 128 rows of L, all of B*S) is AllGather'd per batch.
  - output projection sharded by output feature D: core c computes
    out^T rows [128c, 128c+128) using Wo^T[:, 128c:128c+128] for ALL s.
  - host concatenates the 8 out^T blocks and transposes.

Matmuls in bf16 (fp32 PSUM accumulation), except the Q/K projections which
run fp8e4m3 DoubleRow (2 contraction rows/cell, half the stream cycles —
their quantization only perturbs softmax scores, ~1.1e-2 total rel err).
V/Wo stay bf16 (their element error reaches the output directly). exp on
ScalarE in fp32 with fused free-axis accumulation for softmax denominators;
1/denom is folded into V rows (per-partition scalar) so no full-size
normalization pass. V^T->V transposes ride the DMA xbar (PE is
power-throttled; every PE cycle counts).
"""

import os

import numpy as np
import ml_dtypes

import concourse.bass as bass
import concourse.mybir as mybir
import concourse.tile as tile
from concourse.bass_utils import run_bass_kernel_spmd
from concourse.masks import make_identity

B, S, D = 4, 2048, 1024
L, H = 1024, 16
DH = L // H               # 64
NCORES = 8
LPC = L // NCORES         # 128 l-rows (= 2 heads) per core
DPC = D // NCORES         # 128 out-feature rows per core
SCALE = 1.0 / (D ** 0.5)
KC = S // 128             # 16 key chunks of 128
BF16 = mybir.dt.bfloat16
F32 = mybir.dt.float32
F8 = mybir.dt.float8e4
EXP = mybir.ActivationFunctionType.Exp

LAST_EXEC_NS = None


def _body(tc, xT, x8, wq8, wk8, wvT, woT, outT, zloc, zfull):
    nc = tc.nc
    from contextlib import ExitStack

    with ExitStack() as ctx:
        const = ctx.enter_context(tc.tile_pool(name="const", bufs=1))
        xpool = ctx.enter_context(tc.tile_pool(name="xpool", bufs=1))
        qk = ctx.enter_context(tc.tile_pool(name="qk", bufs=2))
        vtpool = ctx.enter_context(tc.tile_pool(name="vtpool", bufs=1))
        vpool = ctx.enter_context(tc.tile_pool(name="vpool", bufs=2))
        apool = ctx.enter_context(tc.tile_pool(name="apool", bufs=7))
        small = ctx.enter_context(tc.tile_pool(name="small", bufs=8))
        ztp = ctx.enter_context(tc.tile_pool(name="ztp", bufs=2))
        zslab = ctx.enter_context(tc.tile_pool(name="zslab", bufs=2))
        osb_p = ctx.enter_context(tc.tile_pool(name="osb_p", bufs=2))
        # all 8 PSUM banks in one 4-deep [128,1024] pool: scores, AV
        # partials, projections, out-projection all cycle through it
        ps = ctx.enter_context(tc.tile_pool(name="ps", bufs=1, space="PSUM"))

        # ---- constants: weights ----
        # Q/K weights in fp8 (DoubleRow: 2 contraction rows per cell);
        # V/Wo stay bf16 — their element error reaches the output directly.
        wq_sb = const.tile([128, 4, 2, 128], F8, name="wq_sb")
        wk_sb = const.tile([128, 4, 2, 128], F8, name="wk_sb")
        nc.sync.dma_start(wq_sb, wq8)
        nc.sync.dma_start(wk_sb, wk8)
        wv_sb = const.tile([128, 8, 128], BF16, name="wv_sb")
        wo_sb = const.tile([128, 8, 128], BF16, name="wo_sb")
        for dc in range(8):
            nc.sync.dma_start(wv_sb[:, dc, :], wvT[dc * 128:(dc + 1) * 128, :])
            nc.sync.dma_start(wo_sb[:, dc, :], woT[dc * 128:(dc + 1) * 128, :])


        def load_x(b):
            x_c = []
            for dc in range(8):
                xc = xpool.tile([128, S], BF16, name=f"xc{dc}", tag=f"x{dc}")
                nc.gpsimd.dma_start(xc, xT[b, dc * 128:(dc + 1) * 128, :])
                x_c.append(xc)
            for j in range(4):
                xc = xpool.tile([128, 2, S], F8, name=f"x8c{j}", tag=f"x8{j}")
                nc.gpsimd.dma_start(xc, x8[b, j])
                x_c.append(xc)
            return x_c

        def proj_w(w_sb, nm, x_c):
            """V^T projection — bf16, weight-stationary."""
            pool = vtpool if nm == "vt" else qk
            dest = pool.tile([128, S], BF16, name=nm, tag=nm)
            for half in range(2):
                pw = ps.tile([128, 1024], F32, name="pw", tag="work", bufs=4)
                for dc in range(8):
                    for q in range(2):
                        sc = half * 2 + q
                        nc.tensor.matmul(
                            pw[:, q * 512:(q + 1) * 512],
                            lhsT=w_sb[:, dc, :],
                            rhs=x_c[dc][:, sc * 512:(sc + 1) * 512],
                            start=(dc == 0),
                            stop=(dc == 7),
                        )
                nc.vector.tensor_copy(dest[:, half * 1024:(half + 1) * 1024],
                                      pw)
            return dest

        def proj_w8(w_sb, nm, x_c):
            """Q/K projection in fp8 DoubleRow: half the stream cycles."""
            dest = qk.tile([128, S], BF16, name=nm, tag=nm)
            for half in range(2):
                pw = ps.tile([128, 1024], F32, name="pw8", tag="work", bufs=4)
                for j in range(4):
                    for q in range(2):
                        sc = half * 2 + q
                        nc.tensor.matmul(
                            pw[:, q * 512:(q + 1) * 512],
                            lhsT=w_sb[:, j, :, :],
                            rhs=x_c[8 + j][:, :, sc * 512:(sc + 1) * 512],
                            start=(j == 0),
                            stop=(j == 3),
                            perf_mode=mybir.MatmulPerfMode.DoubleRow,
                        )
                nc.vector.tensor_copy(dest[:, half * 1024:(half + 1) * 1024],
                                      pw)
            return dest

        def transpose_v(vt):
            # transpose VT [dh2, s] -> V [s, dh2] in 128-chunks on the DMA
            # xbar: PE is power-throttled, so keep transposes off it
            v_sb = vpool.tile([128, KC, 128], BF16, name="v_sb", tag="v")
            for c in range(KC):
                nc.sync.dma_start_transpose(
                    v_sb[:, c, :], vt[:, c * 128:(c + 1) * 128])
            return v_sb

        def proj(b):
            x_c = load_x(b)
            qt = proj_w8(wq_sb, "qt", x_c)
            kt = proj_w8(wk_sb, "kt", x_c)
            vt = proj_w(wv_sb, "vt", x_c)
            return qt, kt, transpose_v(vt)

        def scores_exp(kc, qt, kt, v_sb):
            """Scores + exp + denominators + scaled V for key-chunk kc.
            Both heads' matmuls are issued adjacently so the K=64 pairs
            co-execute in disjoint PE row-groups."""
            a_ts = [
                apool.tile([128, S], BF16, name=f"a{h}", tag=f"a{h}")
                for h in range(2)
            ]
            accs = [[], []]
            for half in range(2):
                tiles = [
                    ps.tile([128, 1024], F32, name=f"psc{h}", tag="work",
                            bufs=4)
                    for h in range(2)
                ]
                for qq in range(2):
                    q0 = half * 1024 + qq * 512
                    for h in range(2):
                        hp = h * 64
                        nc.tensor.matmul(
                            tiles[h][:, qq * 512:(qq + 1) * 512],
                            lhsT=kt[hp:hp + 64, kc * 128:(kc + 1) * 128],
                            rhs=qt[hp:hp + 64, q0:q0 + 512],
                            start=True,
                            stop=True,
                        )
                for h in range(2):
                    acc = small.tile([128, 1], F32, name="acc", tag="acc")
                    nc.scalar.activation(
                        a_ts[h][:, half * 1024:(half + 1) * 1024],
                        tiles[h],
                        EXP,
                        scale=float(SCALE),
                        accum_out=acc,
                    )
                    accs[h].append(acc)
            res = []
            for h in range(2):
                den = small.tile([128, 1], F32, name="den", tag="den")
                nc.vector.tensor_add(den, accs[h][0], accs[h][1])
                rec = small.tile([128, 1], F32, name="rec", tag="rec")
                nc.vector.reciprocal(rec, den)
                vs = small.tile([128, DH], BF16, name="vs", tag=f"vs{h}")
                nc.vector.tensor_scalar_mul(
                    vs, v_sb[:, kc, h * 64:h * 64 + 64], rec)
                res.append((a_ts[h], vs))
            return res

        def av_pair(units, zac, first):
            """AV for two kc units: dense 16-matmul burst into two PSUM
            tiles (accumulating over the 2 kc), then fold into the SBUF
            f32 accumulator on DVE. Keeps zT out of PSUM so the work pool
            can be 4 deep, and gives PE a long uninterrupted burst."""
            zps = [
                ps.tile([128, 1024], F32, name=f"zp{q2}", tag="work", bufs=4)
                for q2 in range(2)
            ]
            last = len(units) - 1
            for j, (kc, pair) in enumerate(units):
                for qc in range(4):
                    for h in range(2):
                        a_t, vs = pair[h]
                        hp = h * 64
                        nc.tensor.matmul(
                            zps[qc // 2][hp:hp + 64,
                                         (qc % 2) * 512:(qc % 2 + 1) * 512],
                            lhsT=vs,
                            rhs=a_t[:, qc * 512:(qc + 1) * 512],
                            start=(j == 0),
                            stop=(j == last),
                            skip_group_check=True,
                        )
            for q2 in range(2):
                sl = zac[:, q2 * 1024:(q2 + 1) * 1024]
                if first:
                    nc.vector.tensor_copy(sl, zps[q2])
                else:
                    nc.vector.tensor_add(sl, zps[q2], sl)

        def attention(b, cur, nxt_b):
            """Attention for batch b; the NEXT batch's x-load/projections/
            transposes and the PREVIOUS batch's out-projection slab loads
            are issued mid-stream so no engine waits at batch boundaries.
            Returns (next batch's (qt, kt, v) or None, prev outproj tiles)."""
            qt, kt, v_sb = cur
            zac = ztp.tile([128, S], F32, name="zac", tag="zac")
            pending = []
            npairs = 0
            nxt = {}
            prev_tiles = None
            for kc in range(KC):
                pending.append((kc, scores_exp(kc, qt, kt, v_sb)))
                if len(pending) >= 6:
                    av_pair(pending[:2], zac, first=(npairs == 0))
                    pending = pending[2:]
                    npairs += 1
                if kc == 13 and b >= 1:
                    prev_tiles = outproj_load(b - 1)
                if nxt_b is not None:
                    if kc == 4:
                        nxt["x"] = load_x(nxt_b)
                    elif kc == 8:
                        nxt["qt"] = proj_w8(wq_sb, "qt", nxt["x"])
                    elif kc == 10:
                        nxt["kt"] = proj_w8(wk_sb, "kt", nxt["x"])
                    elif kc == 12:
                        nxt["vt"] = proj_w(wv_sb, "vt", nxt["x"])
                    elif kc == 14:
                        nxt["v"] = transpose_v(nxt["vt"])
            while pending:
                av_pair(pending[:2], zac, first=(npairs == 0))
                pending = pending[2:]
                npairs += 1
            # flush + AllGather per s-half (smaller exposed tail; the
            # f32 -> bf16 cast happens inside the SWDGE DMA)
            for half in range(2):
                nc.gpsimd.dma_start(
                    zloc[b, half], zac[:, half * 1024:(half + 1) * 1024])
                nc.gpsimd.collective_compute(
                    "AllGather",
                    mybir.AluOpType.bypass,
                    replica_groups=[list(range(NCORES))],
                    ins=[zloc[b, half].opt()],
                    outs=[zfull[2 * b + half][:, :].opt()],
                )
            nxt_cur = (nxt["qt"], nxt["kt"], nxt["v"]) if nxt_b is not None \
                else None
            return nxt_cur, prev_tiles

        def outproj_load(b):
            """Prefetch the gathered z^T slabs for both halves; issued
            mid-attention of the following batch so the DMA latency (and
            any residual AllGather latency) hides under compute."""
            tiles = []
            for half in range(2):
                zf_c = []
                for j in range(4):
                    zf = zslab.tile([128, 2, S // 2], BF16, name=f"zf{j}",
                                    tag=f"zf{j}")
                    nc.gpsimd.dma_start(
                        zf,
                        zfull[2 * b + half][j * 256:(j + 1) * 256, :]
                        .rearrange("(c p) s -> p c s", p=128),
                    )
                    zf_c.append(zf)
                tiles.append(zf_c)
            return tiles

        def outproj(b, tiles):
            for half in range(2):
                zf_c = tiles[half]
                po = ps.tile([128, 1024], F32, name="po", tag="work", bufs=4)
                for lc in range(8):
                    for sc in range(2):
                        nc.tensor.matmul(
                            po[:, sc * 512:(sc + 1) * 512],
                            lhsT=wo_sb[:, lc, :],
                            rhs=zf_c[lc // 2][:, lc % 2,
                                              sc * 512:(sc + 1) * 512],
                            start=(lc == 0),
                            stop=(lc == 7),
                        )
                o_sb = osb_p.tile([128, S // 2], F32, name="o_sb", tag="osb")
                nc.vector.tensor_copy(o_sb, po)
                nc.sync.dma_start(
                    outT[:, b * S + half * 1024:b * S + (half + 1) * 1024],
                    o_sb)

        cur = proj(0)
        for b in range(B):
            cur, prev_tiles = attention(b, cur, b + 1 if b < B - 1 else None)
            if b >= 1:
                outproj(b - 1, prev_tiles)
        outproj(B - 1, outproj_load(B - 1))


def _legalize_waits(nc):
    """This walrus build accepts only ~2 sync commands (1 wait + 1 inc) per
    instruction for the standard engine/DMA templates; Tile can emit 2-3
    waits (WAR + WAW + RAW). Hoist all but one wait of any multi-wait
    instruction onto single-wait NOPs on the same engine, immediately
    before it — the raw-bass `wait_ge; op` pattern. Drain/EventSemaphore
    templates accept many waits (the kernel-tail barrier relies on it)."""
    import bass_rust

    n = 0
    for f in nc.m.functions:
        for blk in f.blocks:
            out = []
            changed = False
            for inst in blk.instructions:
                si = inst.sync_info
                if si is not None and len(si.on_wait) > 1:
                    for w in si.on_wait[:-1]:
                        n += 1
                        out.append(
                            bass_rust.InstNoOp(
                                name=f"I-hoistwait-{n}",
                                engine=inst.engine,
                                bass_nofuse=True,
                                sync_info=bass_rust.SyncInfo(
                                    on_wait=[w], on_update=[]
                                ),
                            )
                        )
                    inst.sync_info = bass_rust.SyncInfo(
                        on_wait=[si.on_wait[-1]], on_update=list(si.on_update)
                    )
                    changed = True
                out.append(inst)
            if changed:
                blk.instructions = out


def build(legalize=True):
    nc = bass.Bass(
        "TRN2",
        target_bir_lowering=False,
        debug=False,
        enable_asserts=False,
        num_devices=NCORES,
    )
    xT = nc.dram_tensor("xT", [B, D, S], BF16, kind="ExternalInput").ap()
    x8 = nc.dram_tensor("x8", [B, 4, 128, 2, S], F8, kind="ExternalInput").ap()
    wq8 = nc.dram_tensor("wq8", [128, 4, 2, LPC], F8, kind="ExternalInput").ap()
    wk8 = nc.dram_tensor("wk8", [128, 4, 2, LPC], F8, kind="ExternalInput").ap()
    wvT = nc.dram_tensor("wvT", [D, LPC], BF16, kind="ExternalInput").ap()
    woT = nc.dram_tensor("woT", [L, DPC], BF16, kind="ExternalInput").ap()
    outT = nc.dram_tensor("outT", [DPC, B * S], F32, kind="ExternalOutput").ap()

    with tile.TileContext(nc) as tc:
        from contextlib import ExitStack

        with ExitStack() as ctx:
            dram = ctx.enter_context(tc.tile_pool(name="dram", bufs=1, space="DRAM"))
            zloc = dram.tile([B, 2, LPC, S // 2], BF16, name="zloc")
            zfull = [
                dram.tile([L, S // 2], BF16, name=f"zfull{i}", tag=f"zfull{i}",
                          addr_space="Shared")
                for i in range(2 * B)
            ]
            _body(tc, xT, x8, wq8, wk8, wvT, woT, outT, zloc, zfull)
    if legalize:
        # the inserted NOPs are invisible to the simulator's race-detector
        # registry; sim callers pass legalize=False (identical semantics)
        _legalize_waits(nc)
    return nc


def make_in_maps(x, Wq, Wk, Wv, Wo):
    bf = ml_dtypes.bfloat16
    f8 = ml_dtypes.float8_e4m3
    x = np.asarray(x, np.float32)
    xTf = np.ascontiguousarray(x.transpose(0, 2, 1))            # (B, D, S)
    xT = xTf.astype(bf)
    # fp8 copy with D-chunk pairs interleaved for DoubleRow matmuls
    x8 = np.ascontiguousarray(
        xTf.reshape(B, 4, 2, 128, S).transpose(0, 1, 3, 2, 4)).astype(f8)
    WoT = np.ascontiguousarray(np.asarray(Wo, np.float32).T)    # (L, D)

    def w8(W, rs):
        wT = np.asarray(W, np.float32)[rs].T                    # (D, 128)
        return np.ascontiguousarray(
            wT.reshape(4, 2, 128, LPC).transpose(2, 0, 1, 3)).astype(f8)

    in_maps = []
    for c in range(NCORES):
        rs = slice(128 * c, 128 * (c + 1))
        in_maps.append({
            "xT": xT,
            "x8": x8,
            "wq8": w8(Wq, rs),
            "wk8": w8(Wk, rs),
            "wvT": np.ascontiguousarray(np.asarray(Wv, np.float32)[rs].T).astype(bf),
            "woT": np.ascontiguousarray(WoT[:, rs]).astype(bf),
        })
    return in_maps


def _install_ntff_hook_shim():
    """This container's `antenv` lacks `axon_hooks`; recreate the NTFF
    profile hook (same ctypes recipe as trn_agent_boot.trn_boot) so
    run_bass_kernel_spmd(trace=True) can capture exec_time_ns."""
    import sys
    import types
    import ctypes
    import contextlib

    try:
        import antenv.axon_hooks  # noqa: F401
        return
    except ImportError:
        pass

    hook = None
    so_path = os.environ.get("PJRT_LIBRARY_PATH")
    if so_path and os.path.exists(so_path):
        try:
            lib = ctypes.CDLL(so_path)
            if hasattr(lib, "axon_start_nrt_profile"):
                lib.axon_start_nrt_profile.argtypes = [
                    ctypes.POINTER(ctypes.c_int64),
                    ctypes.c_size_t,
                ]
                lib.axon_start_nrt_profile.restype = ctypes.c_int64
                lib.axon_stop_nrt_profile.argtypes = [ctypes.c_char_p]
                lib.axon_stop_nrt_profile.restype = ctypes.c_int64

                @contextlib.contextmanager
                def _hook(output_dir, device_ids):
                    import jax

                    jax.devices()
                    if device_ids:
                        ids = (ctypes.c_int64 * len(device_ids))(*device_ids)
                        rc = lib.axon_start_nrt_profile(ids, len(device_ids))
                    else:
                        rc = lib.axon_start_nrt_profile(None, 0)
                    if rc != 0:
                        raise RuntimeError(f"axon_start_nrt_profile rc={rc}")
                    try:
                        yield
                    finally:
                        n = lib.axon_stop_nrt_profile(str(output_dir).encode())
                        print(f"profile: {n} file(s) written to {output_dir}")

                hook = _hook
        except OSError:
            hook = None

    mod = types.ModuleType("antenv.axon_hooks")
    mod.get_axon_ntff_profile_hook = lambda: hook
    mod.set_axon_ntff_profile_hook = lambda h: None
    sys.modules["antenv.axon_hooks"] = mod
    import antenv

    antenv.axon_hooks = mod


def _gather(res):
    return np.concatenate(
        [np.asarray(res.results[c]["outT"], np.float32) for c in range(NCORES)],
        axis=0,
    )  # (D, B*S)


def kernel(x, Wq, Wk, Wv, Wo):
    global LAST_EXEC_NS
    in_maps = make_in_maps(x, Wq, Wk, Wv, Wo)
    nc = build()
    trace = bool(int(os.environ.get("BASS_KERNEL_TRACE", "0")))
    if trace:
        _install_ntff_hook_shim()
    core_ids = list(range(NCORES))
    # Run twice and cross-check: the first execution of a freshly-loaded
    # NEFF was once observed to produce a corrupted result; a re-run is
    # ~0.6ms of device time against a multi-second compile+load.
    r1 = _gather(run_bass_kernel_spmd(nc, in_maps, core_ids=core_ids))
    res = run_bass_kernel_spmd(nc, in_maps, core_ids=core_ids, trace=trace)
    LAST_EXEC_NS = res.exec_time_ns
    r2 = _gather(res)
    if not np.array_equal(r1, r2):
        r3 = _gather(run_bass_kernel_spmd(nc, in_maps, core_ids=core_ids))
        outT = r3 if np.array_equal(r2, r3) else (
            r1 if np.array_equal(r1, r3) else r2)
    else:
        outT = r2
    return np.ascontiguousarray(outT.T).reshape(B, S, D).astype(np.float32)
